# revision 56
# baseline (speedup 1.0000x reference)
"""Trainium2 Bass kernel for nn_CausePredictor (RGCN + pairwise MLP).

Sharding: data-parallel over the pairwise row index i (dim 1 of the
[B,S,S] output): 8 cores x 25 rows, replicated over B=4.  All per-core
differences are encoded as input DATA (column slices / selection
matrices), so one SPMD program serves all cores.

Math (matching reference.py):
  h   = sum_k Ahat_k.T @ (x[b] @ basis_k) + x[b] @ root + bias
        where Ahat_k[i,j] = sum_r comp[r,k] * A[r,i,j] * inv_deg[r,j]
  u   = h @ W1a   (j-indexed term),  v = h @ W1c  (i-indexed term)
  T   = pe_k @ W1b + pe_v @ W1d                  # [11, 512] table
  h1[b,i,j,:] = u[b,j] + v[b,i] + T[pos(i,j)]
  out = sigmoid(relu(relu(h1) @ W2) @ Wp) * mask

On-chip layout is feature-on-partition: [m(128-chunk), pairs] so both
big GEMMs run without activation transposes.
"""

import sys

sys.path.insert(0, "/opt/trn_rl_repo")

import numpy as np

B, S, D, M, P = 4, 200, 300, 512, 100
NREL, MAXL = 9, 10
NCORES = 8
IPC = S // NCORES  # 25 rows of i per core
NU = IPC // 2 + 1  # 13 units per b: 12x 2-row + 1x 1-row
ROWS = B * NU  # 52 output rows per core
FPC = IPC * S  # 5000 pairs per (b, core)

_prog_cache = {}
NB = B  # batches emitted in stage B (debug knob)
NA = B  # batches emitted in stage A
NPER = 10  # peR blocks per mc
SKIP_DVE = False
SKIP_RELU2 = False


def _rel_adj(s):
    ra = np.arange(s)[None, :] - np.arange(s)[:, None]
    for i in range(s):
        ra[i, i + 1 :] = 1
        num = 1
        for o in range(i - 1, -1, -2):
            ra[i, o] = -num
            if o - 1 >= 0:
                ra[i, o - 1] = -num
            num += 1
        ra[i, :i] = np.maximum(ra[i, :i], -8)  # -(WINDOW+1), WINDOW=7
    return ra


def _pack_k(w, width=None):
    """[K, N] -> [128, ceil(K/128)*N], K chunked onto partitions, zero pad."""
    k, n = w.shape
    nch = (k + 127) // 128
    out = np.zeros((128, nch * n), np.float32)
    for c in range(nch):
        r = min(128, k - c * 128)
        out[:r, c * n : c * n + n] = w[c * 128 : c * 128 + r]
    return out


def _build_program():
    import ml_dtypes  # noqa: F401
    import concourse.tile as tile
    from concourse import bacc, mybir
    from concourse.ap import AP

    f32 = mybir.dt.float32
    f32r = mybir.dt.float32r
    bf16 = mybir.dt.bfloat16
    AF = mybir.ActivationFunctionType
    OP = mybir.AluOpType

    nc = bacc.Bacc()

    SC = 256  # padded storage stride: 200 j-cols | 25 i-slice | 31 zero pad
    SCU = 225  # used columns
    dxT = nc.declare_dram_parameter("xT", [D, B * SC], bf16, isOutput=False)
    dahat = nc.declare_dram_parameter("ahat", [128, 4 * SC], bf16, isOutput=False)
    # fb1 = basis(1800) | root(900)
    dfb1 = nc.declare_dram_parameter("fb1", [128, 2700], bf16, isOutput=False)
    dbias = nc.declare_dram_parameter("bias", [128, 3], f32, isOutput=False)
    # fb2 = w1a(1536) | w1c(1536)
    dfb2 = nc.declare_dram_parameter("fb2", [128, 3072], bf16, isOutput=False)
    # fb4 = w2(2048) | wp(4)
    dfb4 = nc.declare_dram_parameter("fb4", [128, 2052], bf16, isOutput=False)
    # host-computed T-row expansion, [128, 4mc x FPC]
    dperc = nc.declare_dram_parameter("perc", [128, 4 * FPC], bf16, isOutput=False)
    dout = nc.declare_dram_parameter("out", [B, NU * 400], f32, isOutput=True)

    DCW = [128, 128, 44]  # D=300 chunks
    JCW = [128, 72]  # S=200 chunks

    with tile.TileContext(nc) as tc:
        with (
            tc.tile_pool(name="persist", bufs=1) as pp,
            tc.tile_pool(name="work", bufs=3) as pwork,
            tc.tile_pool(name="sigp", bufs=2) as psig,
        ):
            def load(name, shape, dt, src):
                t = pp.tile(shape, dt, tag=name, name=name)
                nc.sync.dma_start(t[:, :], src)
                return t

            class ColView:
                def __init__(self, tile, c0, cast=None):
                    self.t, self.c0, self.cast = tile, c0, cast

                def __getitem__(self, idx):
                    ps, cs = idx
                    ap = self.t[ps, cs.start + self.c0 : cs.stop + self.c0]
                    return ap.bitcast(self.cast) if self.cast else ap

            fb1 = load("fb1", [128, 2700], bf16, dfb1[:, :])
            xT = [load(f"xT{c}", [DCW[c], B * SC], bf16,
                       dxT[c * 128 : c * 128 + DCW[c], :]) for c in range(3)]
            bias_t = load("bias", [128, 3], f32, dbias[:, :])
            ahat = load("ahat", [128, 4 * SC], bf16, dahat[:, :])
            fb2 = load("fb2", [128, 3072], bf16, dfb2[:, :])
            fb4 = load("fb4", [128, 2052], bf16, dfb4[:, :])
            # peRcat in two DMAs: units 0-2 of every mc-block first
            peRcat = pp.tile([128, 4 * FPC], bf16, tag="perc", name="perc")
            hpr = peRcat[:, 0:1].tensor
            hdp = dperc[0:1, 0:1].tensor
            nc.sync.dma_start(
                AP(hpr, 0, [[4 * FPC, 128], [FPC, 4], [1, 1200]]),
                AP(hdp, 0, [[4 * FPC, 128], [FPC, 4], [1, 1200]]))
            nc.sync.dma_start(
                AP(hpr, 1200, [[4 * FPC, 128], [FPC, 4], [1, FPC - 1200]]),
                AP(hdp, 1200, [[4 * FPC, 128], [FPC, 4], [1, FPC - 1200]]))
            basis = ColView(fb1, 0)
            root = ColView(fb1, 1800)
            bias = bias_t
            w1a = ColView(fb2, 0)
            w1c = ColView(fb2, 1536)
            w2 = ColView(fb4, 0)
            wp = ColView(fb4, 2048)

            hT = [[pp.tile([DCW[ec], SCU], bf16, tag=f"hT{b}{ec}", name=f"hT{b}{ec}")
                   for ec in range(3)] for b in range(B)]
            # uTcat[b][:, mc*200 + j] = u[b, j] for the mc-th 128-feature chunk
            uTcat = [pp.tile([128, 800], bf16, tag=f"uTcat{b}", name=f"uTcat{b}")
                     for b in range(B)]
            vT = [[pp.tile([128, IPC], f32, tag=f"vT{b}{mc}", name=f"vT{b}{mc}")
                   for mc in range(4)] for b in range(B)]

            # ---------------- stage A: h/u/v -------------------------------
            with tc.tile_pool(name="psA1", bufs=4, space="PSUM") as psA:
                t1 = [[[pp.tile([JCW[jc], D], bf16, tag=f"t1_{b}{k}{jc}",
                                name=f"t1_{b}{k}{jc}")
                        for jc in range(2)] for k in range(2)] for b in range(B)]
                # phase 1: all t1 groups (b-independent -> PE never stalls)
                for b in range(NA):
                    for k in range(2):
                        for jc in range(2):
                            t1ps = psA.tile([JCW[jc], D], f32, tag="mps", name="t1ps")
                            for dc in range(3):
                                nc.tensor.matmul(
                                    t1ps[:, :],
                                    xT[dc][:, b * SC + jc * 128 : b * SC + jc * 128 + JCW[jc]],
                                    basis[0 : DCW[dc], (k * 3 + dc) * D : (k * 3 + dc) * D + D],
                                    start=(dc == 0), stop=(dc == 2),
                                )
                            if (k * 2 + jc) % 2 == 0:
                                nc.vector.tensor_copy(t1[b][k][jc][:, :], t1ps[:, :])
                            else:
                                nc.scalar.activation(t1[b][k][jc][:, :], t1ps[:, :],
                                                     AF.Copy)
            with tc.tile_pool(name="psA2", bufs=4, space="PSUM") as psA:
                # phase 2: all h groups
                for b in range(NA):
                    for ec in range(3):
                        hps = psA.tile([DCW[ec], SCU], f32, tag="hps", name="hps")
                        first = True
                        for k in range(2):
                            for jc in range(2):
                                nc.tensor.matmul(
                                    hps[:, :],
                                    t1[b][k][jc][:, ec * 128 : ec * 128 + DCW[ec]],
                                    ahat[0 : JCW[jc], (k * 2 + jc) * SC : (k * 2 + jc) * SC + SCU],
                                    start=first, stop=False)
                                first = False
                        for dc in range(3):
                            nc.tensor.matmul(
                                hps[:, :],
                                root[0 : DCW[dc], dc * D + ec * 128 : dc * D + ec * 128 + DCW[ec]],
                                xT[dc][:, b * SC : b * SC + SCU],
                                start=False, stop=(dc == 2))
                        nc.scalar.activation(hT[b][ec][:, :], hps[:, :], AF.Identity,
                                             bias=bias[0 : DCW[ec], ec : ec + 1])
            with tc.tile_pool(name="psA3", bufs=4, space="PSUM") as psA:
                # phase 3: all u/v groups (u padded to 256 free cols for f32r)
                for b in range(NA):
                    for mc in range(4):
                        ups = psA.tile([128, S + IPC], f32, tag="uvps", name="ups")
                        for ec in range(3):
                            nc.tensor.matmul(
                                ups[:, 0:S],
                                w1a[0 : DCW[ec], ec * M + mc * 128 : ec * M + mc * 128 + 128],
                                hT[b][ec][:, 0:S], start=(ec == 0), stop=False)
                        for ec in range(3):
                            nc.tensor.matmul(
                                ups[:, S : S + IPC],
                                w1c[0 : DCW[ec], ec * M + mc * 128 : ec * M + mc * 128 + 128],
                                hT[b][ec][:, S : S + IPC], start=(ec == 0), stop=(ec == 2))
                        nc.scalar.activation(
                            uTcat[b][:, mc * 200 : mc * 200 + 200], ups[:, 0:S],
                            AF.Copy)
                        nc.vector.tensor_copy(vT[b][mc][:, :], ups[:, S : S + IPC])


            # ---------------- stage B: the pairwise MLP ------------------
            with (
                tc.tile_pool(name="ps2", bufs=5, space="PSUM") as ps2,
                tc.tile_pool(name="ps3", bufs=3, space="PSUM") as ps3,
            ):
                def emit_g3(prev):
                    p_b, p_sigb, p_u, p_ncols, p_rh2 = prev
                    g3 = ps3.tile([1, 400], f32, tag="g3", name="g3")
                    for n in range(4):
                        nc.tensor.matmul(
                            g3[:, :p_ncols], wp[0:128, n : n + 1], p_rh2[n][:, :p_ncols],
                            start=(n == 0), stop=(n == 3))
                    nc.scalar.activation(
                        p_sigb[0:1, p_u * 400 : p_u * 400 + p_ncols],
                        g3[:, :p_ncols], AF.Sigmoid)
                    if p_b == NB - 1:
                        # last batch: ship finely so the tail DMA is tiny
                        if p_u % 4 == 3:
                            nc.sync.dma_start(
                                dout[p_b : p_b + 1, (p_u - 3) * 400 : (p_u + 1) * 400],
                                p_sigb[0:1, (p_u - 3) * 400 : (p_u + 1) * 400])
                        if p_u == NU - 1:
                            nc.sync.dma_start(
                                dout[p_b : p_b + 1, 4800 : NU * 400],
                                p_sigb[0:1, 4800 : NU * 400])
                    elif p_u == NU - 1:
                        nc.sync.dma_start(dout[p_b : p_b + 1, :], p_sigb[0:1, :])

                prev = None
                for b in range(NB):
                    sigb = psig.tile([1, NU * 400], f32, tag="sigb", name="sigb")
                    for u in range(NU):
                        nil = 2 if u < NU - 1 else 1
                        ncols = nil * S
                        # rh1cat[:, mc*400 + h*200 + j] = u_j + T[pos] (+v, relu)
                        rh1cat = pwork.tile([128, 1600], bf16, tag="rh1cat",
                                            name="rh1cat")
                        hr = rh1cat[:, 0:1].tensor
                        hu = uTcat[b][:, 0:1].tensor
                        hp = peRcat[:, 0:1].tensor
                        nc.vector.tensor_add(
                            AP(hr, 0, [[1600, 128], [400, 4], [200, nil], [1, 200]]),
                            AP(hu, 0, [[800, 128], [200, 4], [0, nil], [1, 200]]),
                            AP(hp, u * 400,
                               [[4 * FPC, 128], [FPC, 4], [200, nil], [1, 200]]))
                        for mc in range(4):
                            eng = nc.vector if mc < 2 else nc.gpsimd
                            for h in range(nil):
                                eng.tensor_scalar(
                                    out=rh1cat[:, mc * 400 + h * 200 : mc * 400 + h * 200 + 200],
                                    in0=rh1cat[:, mc * 400 + h * 200 : mc * 400 + h * 200 + 200],
                                    scalar1=vT[b][mc][:, 2 * u + h : 2 * u + h + 1],
                                    scalar2=0.0,
                                    op0=OP.add, op1=OP.max)
                        # GEMM2 + relu2 (n0-2 on Act, n3 on Pool)
                        rh2 = [pwork.tile([128, 400], bf16, tag=f"rh2_{n}", name=f"rh2_{n}")
                               for n in range(4)]
                        for n in range(4):
                            ops = ps2.tile([128, 400], f32, tag="ops", name="ops")
                            for mc in range(4):
                                nc.tensor.matmul(
                                    ops[:, :ncols],
                                    w2[0:128, mc * M + n * 128 : mc * M + n * 128 + 128],
                                    rh1cat[:, mc * 400 : mc * 400 + ncols],
                                    start=(mc == 0), stop=(mc == 3))
                            if nil == 2 or n < 2:
                                nc.scalar.activation(rh2[n][:, :ncols],
                                                     ops[:, :ncols], AF.Relu)
                            else:
                                nc.vector.tensor_scalar(
                                    out=rh2[n][:, :ncols], in0=ops[:, :ncols],
                                    scalar1=0.0, scalar2=None, op0=OP.max)
                        # GEMM3 + sigmoid of the PREVIOUS unit (hides relu2 latency)
                        if prev is not None:
                            emit_g3(prev)
                        prev = (b, sigb, u, ncols, rh2)
                    emit_g3(prev)
                    prev = None

    nc.compile()
    return nc


def _host_prep(x, pe_k, pe_v, comp, basis, root, rgcn_bias, W1):
    import ml_dtypes

    ra = _rel_adj(S) % NREL
    onehot = (ra[None, :, :] == np.arange(NREL)[:, None, None]).astype(np.float64)
    deg = onehot.sum(1)
    inv = np.where(deg > 0, 1.0 / np.maximum(deg, 1.0), 0.0)
    anorm = onehot * inv[:, None, :]
    ahat_full = np.einsum("rk,rij->kij", np.asarray(comp, np.float64), anorm)
    ahat_full = ahat_full.astype(np.float32)  # [2, S, S]
    pos = np.clip(np.arange(S)[:, None] - np.arange(S)[None, :] + 1, 0, MAXL)

    x = np.asarray(x, np.float32)
    W1 = np.asarray(W1, np.float32)
    W1a, W1b = W1[:D], W1[D : D + P]
    W1c, W1d = W1[D + P : 2 * D + P], W1[2 * D + P :]

    bias_p = np.zeros((128, 3), np.float32)
    rb = np.asarray(rgcn_bias, np.float32)
    for c in range(3):
        r = min(128, D - c * 128)
        bias_p[:r, c] = rb[c * 128 : c * 128 + r]

    basis_p = np.concatenate(
        [_pack_k(np.asarray(basis[k], np.float32)) for k in range(2)], axis=1)
    w2_p = np.ascontiguousarray(
        np.asarray(W2_GLOBAL, np.float32).reshape(4, 128, M)
        .transpose(1, 0, 2).reshape(128, 4 * M)).astype(ml_dtypes.bfloat16)
    wp_p = np.ascontiguousarray(np.asarray(WP_GLOBAL, np.float32)[:, 0]
                                .reshape(4, 128).T).astype(ml_dtypes.bfloat16)
    com = {
        "fb1": np.ascontiguousarray(np.concatenate(
            [basis_p, _pack_k(np.asarray(root, np.float32))],
            axis=1)).astype(ml_dtypes.bfloat16),
        "bias": bias_p,
        "fb2": np.ascontiguousarray(np.concatenate(
            [_pack_k(W1a), _pack_k(W1c)], axis=1)).astype(ml_dtypes.bfloat16),
        "fb4": np.ascontiguousarray(np.concatenate([w2_p, wp_p], axis=1)),
    }
    Ttab = (np.asarray(pe_k, np.float64) @ np.asarray(W1b, np.float64)
            + np.asarray(pe_v, np.float64) @ np.asarray(W1d, np.float64))
    Ttab = Ttab.astype(np.float32)  # [11, 512]


    SC = 256  # padded layout: 200 j | 25 i-slice | 31 zeros
    xt_all = x.transpose(2, 0, 1)  # [D, B, S]
    per_core = []
    for c in range(NCORES):
        i0 = c * IPC
        m = dict(com)
        xtc = np.zeros((D, B * SC), np.float32)
        for b in range(B):
            xtc[:, b * SC : b * SC + S] = xt_all[:, b, :]
            xtc[:, b * SC + S : b * SC + S + IPC] = xt_all[:, b, i0 : i0 + IPC]
        m["xT"] = xtc.astype(ml_dtypes.bfloat16)
        ah = np.zeros((128, 4 * SC), np.float32)
        for k in range(2):
            for jc in range(2):
                r = 128 if jc == 0 else 72
                base = (k * 2 + jc) * SC
                ah[:r, base : base + S] = ahat_full[k, jc * 128 : jc * 128 + r, :]
                ah[:r, base + S : base + S + IPC] = ahat_full[k, jc * 128 : jc * 128 + r, i0 : i0 + IPC]
        m["ahat"] = ah.astype(ml_dtypes.bfloat16)
        Tpos = Ttab[pos[i0 : i0 + IPC, :]].reshape(FPC, 4, 128)
        m["perc"] = np.ascontiguousarray(
            Tpos.transpose(2, 1, 0).reshape(128, 4 * FPC)).astype(ml_dtypes.bfloat16)
        per_core.append(m)
    return per_core


W2_GLOBAL = None
WP_GLOBAL = None


def kernel(x, mask, pe_k, pe_v, comp, basis, root, rgcn_bias, W1, W2, Wp,
           _want_results=False, _trace=False):
    global W2_GLOBAL, WP_GLOBAL
    W2_GLOBAL, WP_GLOBAL = W2, Wp

    from concourse.bass_utils import run_bass_kernel_spmd

    if "nc" not in _prog_cache:
        _prog_cache["nc"] = _build_program()
    nc = _prog_cache["nc"]

    in_maps = _host_prep(x, pe_k, pe_v, comp, basis, root, rgcn_bias, W1)
    res = run_bass_kernel_spmd(nc, in_maps, core_ids=list(range(NCORES)),
                               trace=_trace)

    out = np.zeros((B, S, S), np.float32)
    for c in range(NCORES):
        i0 = c * IPC
        rows = res.results[c]["out"].reshape(B, NU, 400)
        for b in range(B):
            for u in range(NU - 1):
                out[b, i0 + 2 * u, :] = rows[b, u, :S]
                out[b, i0 + 2 * u + 1, :] = rows[b, u, S:]
            out[b, i0 + IPC - 1, :] = rows[b, NU - 1, :S]
    out *= np.asarray(mask, np.float32)
    if _want_results:
        return out, res
    return out



# revision 57
# speedup vs baseline: 1.0035x; 1.0035x over previous
"""Trainium2 Bass kernel for nn_CausePredictor (RGCN + pairwise MLP).

Sharding: data-parallel over the pairwise row index i (dim 1 of the
[B,S,S] output): 8 cores x 25 rows, replicated over B=4.  All per-core
differences are encoded as input DATA (column slices / selection
matrices), so one SPMD program serves all cores.

Math (matching reference.py):
  h   = sum_k Ahat_k.T @ (x[b] @ basis_k) + x[b] @ root + bias
        where Ahat_k[i,j] = sum_r comp[r,k] * A[r,i,j] * inv_deg[r,j]
  u   = h @ W1a   (j-indexed term),  v = h @ W1c  (i-indexed term)
  T   = pe_k @ W1b + pe_v @ W1d                  # [11, 512] table
  h1[b,i,j,:] = u[b,j] + v[b,i] + T[pos(i,j)]
  out = sigmoid(relu(relu(h1) @ W2) @ Wp) * mask

On-chip layout is feature-on-partition: [m(128-chunk), pairs] so both
big GEMMs run without activation transposes.
"""

import sys

sys.path.insert(0, "/opt/trn_rl_repo")

import numpy as np

B, S, D, M, P = 4, 200, 300, 512, 100
NREL, MAXL = 9, 10
NCORES = 8
IPC = S // NCORES  # 25 rows of i per core
NU = IPC // 2 + 1  # 13 units per b: 12x 2-row + 1x 1-row
ROWS = B * NU  # 52 output rows per core
FPC = IPC * S  # 5000 pairs per (b, core)

_prog_cache = {}
NB = B  # batches emitted in stage B (debug knob)
NA = B  # batches emitted in stage A
NPER = 10  # peR blocks per mc
SKIP_DVE = False
SKIP_RELU2 = False


def _rel_adj(s):
    ra = np.arange(s)[None, :] - np.arange(s)[:, None]
    for i in range(s):
        ra[i, i + 1 :] = 1
        num = 1
        for o in range(i - 1, -1, -2):
            ra[i, o] = -num
            if o - 1 >= 0:
                ra[i, o - 1] = -num
            num += 1
        ra[i, :i] = np.maximum(ra[i, :i], -8)  # -(WINDOW+1), WINDOW=7
    return ra


def _pack_k(w, width=None):
    """[K, N] -> [128, ceil(K/128)*N], K chunked onto partitions, zero pad."""
    k, n = w.shape
    nch = (k + 127) // 128
    out = np.zeros((128, nch * n), np.float32)
    for c in range(nch):
        r = min(128, k - c * 128)
        out[:r, c * n : c * n + n] = w[c * 128 : c * 128 + r]
    return out


def _build_program():
    import ml_dtypes  # noqa: F401
    import concourse.tile as tile
    from concourse import bacc, mybir
    from concourse.ap import AP

    f32 = mybir.dt.float32
    f32r = mybir.dt.float32r
    bf16 = mybir.dt.bfloat16
    AF = mybir.ActivationFunctionType
    OP = mybir.AluOpType

    nc = bacc.Bacc()

    SC = 256  # padded storage stride: 200 j-cols | 25 i-slice | 31 zero pad
    SCU = 225  # used columns
    dxT = nc.declare_dram_parameter("xT", [D, B * SC], bf16, isOutput=False)
    dahat = nc.declare_dram_parameter("ahat", [128, 4 * SC], bf16, isOutput=False)
    # fb1 = basis(1800) | root(900)
    dfb1 = nc.declare_dram_parameter("fb1", [128, 2700], bf16, isOutput=False)
    dbias = nc.declare_dram_parameter("bias", [128, 3], f32, isOutput=False)
    # fb2 = w1a(1536) | w1c(1536)
    dfb2 = nc.declare_dram_parameter("fb2", [128, 3072], bf16, isOutput=False)
    # fb4 = w2(2048) | wp(4)
    dfb4 = nc.declare_dram_parameter("fb4", [128, 2052], bf16, isOutput=False)
    # host-computed T-row expansion, [128, 4mc x FPC]
    dperc = nc.declare_dram_parameter("perc", [128, 4 * FPC], bf16, isOutput=False)
    dout = nc.declare_dram_parameter("out", [B, NU * 400], f32, isOutput=True)

    DCW = [128, 128, 44]  # D=300 chunks
    JCW = [128, 72]  # S=200 chunks

    with tile.TileContext(nc) as tc:
        with (
            tc.tile_pool(name="persist", bufs=1) as pp,
            tc.tile_pool(name="work", bufs=3) as pwork,
            tc.tile_pool(name="sigp", bufs=2) as psig,
        ):
            def load(name, shape, dt, src):
                t = pp.tile(shape, dt, tag=name, name=name)
                nc.sync.dma_start(t[:, :], src)
                return t

            class ColView:
                def __init__(self, tile, c0, cast=None):
                    self.t, self.c0, self.cast = tile, c0, cast

                def __getitem__(self, idx):
                    ps, cs = idx
                    ap = self.t[ps, cs.start + self.c0 : cs.stop + self.c0]
                    return ap.bitcast(self.cast) if self.cast else ap

            fb1 = load("fb1", [128, 2700], bf16, dfb1[:, :])
            xT = [load(f"xT{c}", [DCW[c], B * SC], bf16,
                       dxT[c * 128 : c * 128 + DCW[c], :]) for c in range(3)]
            bias_t = load("bias", [128, 3], f32, dbias[:, :])
            ahat = load("ahat", [128, 4 * SC], bf16, dahat[:, :])
            fb2 = load("fb2", [128, 3072], bf16, dfb2[:, :])
            fb4 = load("fb4", [128, 2052], bf16, dfb4[:, :])
            peRcat = load("perc", [128, 4 * FPC], bf16, dperc[:, :])
            basis = ColView(fb1, 0)
            root = ColView(fb1, 1800)
            bias = bias_t
            w1a = ColView(fb2, 0)
            w1c = ColView(fb2, 1536)
            w2 = ColView(fb4, 0)
            wp = ColView(fb4, 2048)

            hT = [[pp.tile([DCW[ec], SCU], bf16, tag=f"hT{b}{ec}", name=f"hT{b}{ec}")
                   for ec in range(3)] for b in range(B)]
            # uTcat[b][:, mc*200 + j] = u[b, j] for the mc-th 128-feature chunk
            uTcat = [pp.tile([128, 800], bf16, tag=f"uTcat{b}", name=f"uTcat{b}")
                     for b in range(B)]
            vT = [[pp.tile([128, IPC], f32, tag=f"vT{b}{mc}", name=f"vT{b}{mc}")
                   for mc in range(4)] for b in range(B)]

            # ---------------- stage A: h/u/v -------------------------------
            with tc.tile_pool(name="psA1", bufs=4, space="PSUM") as psA:
                t1 = [[[pp.tile([JCW[jc], D], bf16, tag=f"t1_{b}{k}{jc}",
                                name=f"t1_{b}{k}{jc}")
                        for jc in range(2)] for k in range(2)] for b in range(B)]
                # phase 1: all t1 groups (b-independent -> PE never stalls)
                for b in range(NA):
                    for k in range(2):
                        for jc in range(2):
                            t1ps = psA.tile([JCW[jc], D], f32, tag="mps", name="t1ps")
                            for dc in range(3):
                                nc.tensor.matmul(
                                    t1ps[:, :],
                                    xT[dc][:, b * SC + jc * 128 : b * SC + jc * 128 + JCW[jc]],
                                    basis[0 : DCW[dc], (k * 3 + dc) * D : (k * 3 + dc) * D + D],
                                    start=(dc == 0), stop=(dc == 2),
                                )
                            if (k * 2 + jc) % 2 == 0:
                                nc.vector.tensor_copy(t1[b][k][jc][:, :], t1ps[:, :])
                            else:
                                nc.scalar.activation(t1[b][k][jc][:, :], t1ps[:, :],
                                                     AF.Copy)
            with tc.tile_pool(name="psA2", bufs=4, space="PSUM") as psA:
                # phase 2: all h groups
                for b in range(NA):
                    for ec in range(3):
                        hps = psA.tile([DCW[ec], SCU], f32, tag="hps", name="hps")
                        first = True
                        for k in range(2):
                            for jc in range(2):
                                nc.tensor.matmul(
                                    hps[:, :],
                                    t1[b][k][jc][:, ec * 128 : ec * 128 + DCW[ec]],
                                    ahat[0 : JCW[jc], (k * 2 + jc) * SC : (k * 2 + jc) * SC + SCU],
                                    start=first, stop=False)
                                first = False
                        for dc in range(3):
                            nc.tensor.matmul(
                                hps[:, :],
                                root[0 : DCW[dc], dc * D + ec * 128 : dc * D + ec * 128 + DCW[ec]],
                                xT[dc][:, b * SC : b * SC + SCU],
                                start=False, stop=(dc == 2))
                        nc.scalar.activation(hT[b][ec][:, :], hps[:, :], AF.Identity,
                                             bias=bias[0 : DCW[ec], ec : ec + 1])
            with tc.tile_pool(name="psA3", bufs=4, space="PSUM") as psA:
                # phase 3: all u/v groups (u padded to 256 free cols for f32r)
                for b in range(NA):
                    for mc in range(4):
                        ups = psA.tile([128, S + IPC], f32, tag="uvps", name="ups")
                        for ec in range(3):
                            nc.tensor.matmul(
                                ups[:, 0:S],
                                w1a[0 : DCW[ec], ec * M + mc * 128 : ec * M + mc * 128 + 128],
                                hT[b][ec][:, 0:S], start=(ec == 0), stop=False)
                        for ec in range(3):
                            nc.tensor.matmul(
                                ups[:, S : S + IPC],
                                w1c[0 : DCW[ec], ec * M + mc * 128 : ec * M + mc * 128 + 128],
                                hT[b][ec][:, S : S + IPC], start=(ec == 0), stop=(ec == 2))
                        nc.scalar.activation(
                            uTcat[b][:, mc * 200 : mc * 200 + 200], ups[:, 0:S],
                            AF.Copy)
                        nc.vector.tensor_copy(vT[b][mc][:, :], ups[:, S : S + IPC])


            # ---------------- stage B: the pairwise MLP ------------------
            with (
                tc.tile_pool(name="ps2", bufs=5, space="PSUM") as ps2,
                tc.tile_pool(name="ps3", bufs=3, space="PSUM") as ps3,
            ):
                def emit_g3(prev):
                    p_b, p_sigb, p_u, p_ncols, p_rh2 = prev
                    g3 = ps3.tile([1, 400], f32, tag="g3", name="g3")
                    for n in range(4):
                        nc.tensor.matmul(
                            g3[:, :p_ncols], wp[0:128, n : n + 1], p_rh2[n][:, :p_ncols],
                            start=(n == 0), stop=(n == 3))
                    nc.scalar.activation(
                        p_sigb[0:1, p_u * 400 : p_u * 400 + p_ncols],
                        g3[:, :p_ncols], AF.Sigmoid)
                    if p_b == NB - 1:
                        # last batch: ship finely so the tail DMA is tiny
                        if p_u % 4 == 3:
                            nc.sync.dma_start(
                                dout[p_b : p_b + 1, (p_u - 3) * 400 : (p_u + 1) * 400],
                                p_sigb[0:1, (p_u - 3) * 400 : (p_u + 1) * 400])
                        if p_u == NU - 1:
                            nc.sync.dma_start(
                                dout[p_b : p_b + 1, 4800 : NU * 400],
                                p_sigb[0:1, 4800 : NU * 400])
                    elif p_u == NU - 1:
                        nc.sync.dma_start(dout[p_b : p_b + 1, :], p_sigb[0:1, :])

                prev = None
                for b in range(NB):
                    sigb = psig.tile([1, NU * 400], f32, tag="sigb", name="sigb")
                    for u in range(NU):
                        nil = 2 if u < NU - 1 else 1
                        ncols = nil * S
                        # rh1cat[:, mc*400 + h*200 + j] = u_j + T[pos] (+v, relu)
                        rh1cat = pwork.tile([128, 1600], bf16, tag="rh1cat",
                                            name="rh1cat")
                        hr = rh1cat[:, 0:1].tensor
                        hu = uTcat[b][:, 0:1].tensor
                        hp = peRcat[:, 0:1].tensor
                        nc.vector.tensor_add(
                            AP(hr, 0, [[1600, 128], [400, 4], [200, nil], [1, 200]]),
                            AP(hu, 0, [[800, 128], [200, 4], [0, nil], [1, 200]]),
                            AP(hp, u * 400,
                               [[4 * FPC, 128], [FPC, 4], [200, nil], [1, 200]]))
                        for mc in range(4):
                            eng = nc.vector if mc < 2 else nc.gpsimd
                            for h in range(nil):
                                eng.tensor_scalar(
                                    out=rh1cat[:, mc * 400 + h * 200 : mc * 400 + h * 200 + 200],
                                    in0=rh1cat[:, mc * 400 + h * 200 : mc * 400 + h * 200 + 200],
                                    scalar1=vT[b][mc][:, 2 * u + h : 2 * u + h + 1],
                                    scalar2=0.0,
                                    op0=OP.add, op1=OP.max)
                        # GEMM2 + relu2 (n0-2 on Act, n3 on Pool)
                        rh2 = [pwork.tile([128, 400], bf16, tag=f"rh2_{n}", name=f"rh2_{n}")
                               for n in range(4)]
                        for n in range(4):
                            ops = ps2.tile([128, 400], f32, tag="ops", name="ops")
                            for mc in range(4):
                                nc.tensor.matmul(
                                    ops[:, :ncols],
                                    w2[0:128, mc * M + n * 128 : mc * M + n * 128 + 128],
                                    rh1cat[:, mc * 400 : mc * 400 + ncols],
                                    start=(mc == 0), stop=(mc == 3))
                            if nil == 2 or n < 2:
                                nc.scalar.activation(rh2[n][:, :ncols],
                                                     ops[:, :ncols], AF.Relu)
                            else:
                                nc.vector.tensor_scalar(
                                    out=rh2[n][:, :ncols], in0=ops[:, :ncols],
                                    scalar1=0.0, scalar2=None, op0=OP.max)
                        # GEMM3 + sigmoid of the PREVIOUS unit (hides relu2 latency)
                        if prev is not None:
                            emit_g3(prev)
                        prev = (b, sigb, u, ncols, rh2)
                    emit_g3(prev)
                    prev = None

    nc.compile()
    return nc


def _host_prep(x, pe_k, pe_v, comp, basis, root, rgcn_bias, W1):
    import ml_dtypes

    ra = _rel_adj(S) % NREL
    onehot = (ra[None, :, :] == np.arange(NREL)[:, None, None]).astype(np.float64)
    deg = onehot.sum(1)
    inv = np.where(deg > 0, 1.0 / np.maximum(deg, 1.0), 0.0)
    anorm = onehot * inv[:, None, :]
    ahat_full = np.einsum("rk,rij->kij", np.asarray(comp, np.float64), anorm)
    ahat_full = ahat_full.astype(np.float32)  # [2, S, S]
    pos = np.clip(np.arange(S)[:, None] - np.arange(S)[None, :] + 1, 0, MAXL)

    x = np.asarray(x, np.float32)
    W1 = np.asarray(W1, np.float32)
    W1a, W1b = W1[:D], W1[D : D + P]
    W1c, W1d = W1[D + P : 2 * D + P], W1[2 * D + P :]

    bias_p = np.zeros((128, 3), np.float32)
    rb = np.asarray(rgcn_bias, np.float32)
    for c in range(3):
        r = min(128, D - c * 128)
        bias_p[:r, c] = rb[c * 128 : c * 128 + r]

    basis_p = np.concatenate(
        [_pack_k(np.asarray(basis[k], np.float32)) for k in range(2)], axis=1)
    w2_p = np.ascontiguousarray(
        np.asarray(W2_GLOBAL, np.float32).reshape(4, 128, M)
        .transpose(1, 0, 2).reshape(128, 4 * M)).astype(ml_dtypes.bfloat16)
    wp_p = np.ascontiguousarray(np.asarray(WP_GLOBAL, np.float32)[:, 0]
                                .reshape(4, 128).T).astype(ml_dtypes.bfloat16)
    com = {
        "fb1": np.ascontiguousarray(np.concatenate(
            [basis_p, _pack_k(np.asarray(root, np.float32))],
            axis=1)).astype(ml_dtypes.bfloat16),
        "bias": bias_p,
        "fb2": np.ascontiguousarray(np.concatenate(
            [_pack_k(W1a), _pack_k(W1c)], axis=1)).astype(ml_dtypes.bfloat16),
        "fb4": np.ascontiguousarray(np.concatenate([w2_p, wp_p], axis=1)),
    }
    Ttab = (np.asarray(pe_k, np.float64) @ np.asarray(W1b, np.float64)
            + np.asarray(pe_v, np.float64) @ np.asarray(W1d, np.float64))
    Ttab = Ttab.astype(np.float32)  # [11, 512]


    SC = 256  # padded layout: 200 j | 25 i-slice | 31 zeros
    xt_all = x.transpose(2, 0, 1)  # [D, B, S]
    per_core = []
    for c in range(NCORES):
        i0 = c * IPC
        m = dict(com)
        xtc = np.zeros((D, B * SC), np.float32)
        for b in range(B):
            xtc[:, b * SC : b * SC + S] = xt_all[:, b, :]
            xtc[:, b * SC + S : b * SC + S + IPC] = xt_all[:, b, i0 : i0 + IPC]
        m["xT"] = xtc.astype(ml_dtypes.bfloat16)
        ah = np.zeros((128, 4 * SC), np.float32)
        for k in range(2):
            for jc in range(2):
                r = 128 if jc == 0 else 72
                base = (k * 2 + jc) * SC
                ah[:r, base : base + S] = ahat_full[k, jc * 128 : jc * 128 + r, :]
                ah[:r, base + S : base + S + IPC] = ahat_full[k, jc * 128 : jc * 128 + r, i0 : i0 + IPC]
        m["ahat"] = ah.astype(ml_dtypes.bfloat16)
        Tpos = Ttab[pos[i0 : i0 + IPC, :]].reshape(FPC, 4, 128)
        m["perc"] = np.ascontiguousarray(
            Tpos.transpose(2, 1, 0).reshape(128, 4 * FPC)).astype(ml_dtypes.bfloat16)
        per_core.append(m)
    return per_core


W2_GLOBAL = None
WP_GLOBAL = None


def kernel(x, mask, pe_k, pe_v, comp, basis, root, rgcn_bias, W1, W2, Wp,
           _want_results=False, _trace=False):
    global W2_GLOBAL, WP_GLOBAL
    W2_GLOBAL, WP_GLOBAL = W2, Wp

    from concourse.bass_utils import run_bass_kernel_spmd

    if "nc" not in _prog_cache:
        _prog_cache["nc"] = _build_program()
    nc = _prog_cache["nc"]

    in_maps = _host_prep(x, pe_k, pe_v, comp, basis, root, rgcn_bias, W1)
    res = run_bass_kernel_spmd(nc, in_maps, core_ids=list(range(NCORES)),
                               trace=_trace)

    out = np.zeros((B, S, S), np.float32)
    for c in range(NCORES):
        i0 = c * IPC
        rows = res.results[c]["out"].reshape(B, NU, 400)
        for b in range(B):
            for u in range(NU - 1):
                out[b, i0 + 2 * u, :] = rows[b, u, :S]
                out[b, i0 + 2 * u + 1, :] = rows[b, u, S:]
            out[b, i0 + IPC - 1, :] = rows[b, NU - 1, :S]
    out *= np.asarray(mask, np.float32)
    if _want_results:
        return out, res
    return out

